# revision 14
# baseline (speedup 1.0000x reference)
"""CRF loss kernel for Trainium2 (8 NeuronCores, data-parallel over batch).

Problem (hardcoded shapes): scores [B=128, T=256, K=64, K=64] f32,
targets [128, 256] int (flattened from_tag*K + to_tag), lengths [128] int.

loss = (sum_b fs[b, END] - gold) / B  where fs is the CRF forward
(log-domain) scan and gold is the gathered gold-path score.

Strategy (per core, 16 batch rows):
  * Linear-domain scan with the per-step 1/C normalizer (C = 128)
    folded into the exp: E'_t = exp(sc_t - log C), a_t = E'_t^T a_{t-1},
    so log alpha_tau = log a_tau + tau*log C.
  * exp runs on ScalarE (f32 -> bf16); the kf-contraction runs on
    TensorE as 8 matmuls per step: lhsT [128, 64] bf16 stacks the pair
    (row 2j on partitions 0-63, row 2j+1 on 64-127); rhs [128, 2] holds
    the staggered previous state; out lands on psum partitions 0-63
    with one column per row, so the psum -> state writeback is just
    2 strided copies per step.
  * The full state history a_t lives in SBUF ([128, T*16] bf16, col
    16*t + r = row r, even rows on partitions 0-63, odd on 64-127) and
    is streamed to DRAM chunk-by-chunk; the host reads a_{L_b-1} per
    row and finishes with log + offsets.
  * Rows freeze at t >= L_b, so score chunks entirely past a row's
    length are never needed: each strip DMA carries a cond= flag
    (host-computed, loaded into engine registers) and is skipped at
    runtime. Skipped chunks leave stale-but-finite data in the strip
    buffer; the garbage results stay confined to that row's psum/state
    column, which the host never reads past L_b - 1. Chunk 0 always
    loads (L >= 1), so buffers are never read uninitialized.
  * Rows are assigned to cores by greedy length-balancing so per-core
    DMA loads (and thus the max-core exec time) are even.
  * Strip DMAs are spread across all three DMA rings (sync HWDGE,
    scalar HWDGE, gpsimd SWDGE); chunk widths ramp [8, 8, 16, 32...]
    so the scan starts after ~2 MB instead of ~8 MB.
  * gold: indirect DMA element-gather of scores[b,t,kf*,kto*], invalid
    positions skipped via an OOB sentinel index, reduced on DVE and
    summed on host.
"""

import math

import numpy as np

import concourse.bacc as bacc
import concourse.bass as bass
import concourse.tile as tile
from concourse import mybir
from concourse.bass_utils import run_bass_kernel_spmd

F32 = mybir.dt.float32
BF16 = mybir.dt.bfloat16
I32 = mybir.dt.int32

B = 128
T = 256
K = 64
START = 62
END = 63
NCORES = 8
BL = B // NCORES          # 16 local batch rows per core
NPAIR = BL // 2           # 8 row-pairs
WMAX = 32                 # max timesteps per DMA chunk
WE = 16                   # timesteps per exp tile
CHUNKS = [8, 8, 16] + [32] * 7          # widths, sum = 256
STARTS = [sum(CHUNKS[:i]) for i in range(len(CHUNKS))]
NCHUNK = len(CHUNKS)
G = BL * T // 128         # gold gather indices per partition (32)
LOG_C = 7.0 * math.log(2.0)  # log(128); E' = exp(sc - LOG_C)


def _build_nc():
    nc = bacc.Bacc("TRN2", target_bir_lowering=False)

    sc = nc.dram_tensor("scores", [BL, T, K, K], F32, kind="ExternalInput")
    gidx = nc.dram_tensor("gidx", [128, G], I32, kind="ExternalInput")
    flags = nc.dram_tensor(
        "flags", [1, NCHUNK * NPAIR], I32, kind="ExternalInput"
    )
    states = nc.dram_tensor("states", [128, T * BL], BF16, kind="ExternalOutput")
    gold = nc.dram_tensor("gold", [128, 1], F32, kind="ExternalOutput")

    eng_enum = {
        "sync": mybir.EngineType.SP,
        "scalar": mybir.EngineType.Activation,
        "gpsimd": mybir.EngineType.Pool,
    }

    with tile.TileContext(nc) as tc:
        with (
            tc.tile_pool(name="strips", bufs=2) as strips,
            tc.tile_pool(name="etiles", bufs=2) as etiles,
            tc.tile_pool(name="persist", bufs=1) as persist,
            tc.tile_pool(name="pers_psum", bufs=1, space="PSUM") as pers_psum,
        ):
            # ---- persistent state history -----------------------------
            st = persist.tile([128, T * BL], BF16, tag="st", name="st")
            nc.vector.memset(st[:], 0.0)

            # per-partition bias feeding exp(sc - log C)
            biasc = persist.tile([128, 1], F32, tag="biasc", name="biasc")
            nc.vector.memset(biasc[:], -LOG_C)

            # per-(pair, chunk) DMA-enable flags, chunk-major
            flsb = persist.tile([1, NCHUNK * NPAIR], I32, tag="flsb", name="flsb")
            nc.sync.dma_start(out=flsb[:], in_=flags[:])

            psum_bufs = [
                pers_psum.tile([K, BL], F32, tag=f"ps{i}", name=f"ps{i}")
                for i in range(2)
            ]

            # ---- gold gather (runs concurrently with the scan) --------
            idxs = persist.tile([128, G], I32, tag="idxs", name="idxs")
            gath = persist.tile([128, G], F32, tag="gath", name="gath")
            goldsb = persist.tile([128, 1], F32, tag="goldsb", name="goldsb")
            nc.gpsimd.dma_start(out=idxs[:], in_=gidx[:])
            nc.gpsimd.memset(gath[:], 0.0)
            sc_flat = sc[:].rearrange(
                "b t kf (kto one) -> (b t kf kto) one", one=1
            )
            nc.gpsimd.indirect_dma_start(
                out=gath[:],
                out_offset=None,
                in_=sc_flat,
                in_offset=bass.IndirectOffsetOnAxis(ap=idxs[:], axis=0),
                bounds_check=BL * T * K * K - 1,
                oob_is_err=False,
            )
            nc.vector.tensor_reduce(
                goldsb[:],
                gath[:],
                axis=mybir.AxisListType.XYZW,
                op=mybir.AluOpType.add,
            )
            nc.sync.dma_start(out=gold[:], in_=goldsb[:])

            # ---- init: a_0 = exp(scores[b, 0, START, :]) --------------
            staging = persist.tile([K, BL], F32, tag="staging", name="staging")
            stg16 = persist.tile([K, BL], BF16, tag="stg16", name="stg16")
            nc.sync.dma_start(
                out=staging[:],
                in_=sc[:, 0, START, :].rearrange("b k -> k b"),
            )
            nc.scalar.activation(
                stg16[:], staging[:], mybir.ActivationFunctionType.Exp
            )
            nc.vector.tensor_copy(st[0:64, 0:BL:2], stg16[:, 0:BL:2])
            nc.vector.tensor_copy(st[64:128, 1:BL:2], stg16[:, 1:BL:2])

            # ---- main scan --------------------------------------------
            for c, (wc, sc0) in enumerate(zip(CHUNKS, STARTS)):
                # load 8 pair-strips as 16 conditional half-DMAs over
                # 3 DMA rings (skipped entirely for finished rows)
                conds = {}
                for ename, jlo in (("sync", 0), ("scalar", 4)):
                    _, vals = nc.values_load_multi_w_load_instructions(
                        flsb[0:1, c * NPAIR + jlo : c * NPAIR + jlo + 4],
                        engines=(eng_enum[ename],),
                        min_val=0,
                        max_val=1,
                        skip_runtime_bounds_check=True,
                    )
                    for jj in range(4):
                        conds[jlo + jj] = vals[jj]
                cur = []
                for j in range(NPAIR):
                    s = strips.tile([128, WMAX * K], F32, tag=f"strip{j}")
                    ename = "sync" if j < 4 else "scalar"
                    eng = getattr(nc, ename)
                    for h in range(2):
                        eng.dma_start(
                            out=s[64 * h : 64 * h + 64, 0 : wc * K].rearrange(
                                "p (t k) -> p t k", t=wc
                            ),
                            in_=sc[2 * j + h, sc0 : sc0 + wc]
                            .rearrange("t kf kto -> kf t kto"),
                            cond=conds[j],
                        )
                    cur.append(s)

                # exp to bf16 at (up to) WE-timestep granularity
                nq = (wc + WE - 1) // WE
                etl = []
                for j in range(NPAIR):
                    row = []
                    for q in range(nq):
                        we = min(WE, wc - q * WE)
                        e = etiles.tile([128, WE * K], BF16, tag=f"e{j}_{q}")
                        nc.scalar.activation(
                            e[:, 0 : we * K],
                            cur[j][:, q * WE * K : (q * WE + we) * K],
                            mybir.ActivationFunctionType.Exp,
                            bias=biasc[:],
                        )
                        row.append(e)
                    etl.append(row)

                for tl in range(wc):
                    t = sc0 + tl
                    if t == 0:
                        continue
                    q, tq = tl // WE, tl % WE
                    ps = psum_bufs[t % 2]
                    prev = st[:, (t - 1) * BL : t * BL]
                    for j in range(NPAIR):
                        nc.tensor.matmul(
                            out=ps[:, 2 * j : 2 * j + 2],
                            lhsT=etl[j][q][:, tq * K : (tq + 1) * K],
                            rhs=prev[:, 2 * j : 2 * j + 2],
                            start=True,
                            stop=True,
                        )
                    # state col 2j   = row 2j   (psum top -> top)
                    # state col 2j+1 = row 2j+1 (psum top -> bottom)
                    new = st[:, t * BL : (t + 1) * BL]
                    nc.vector.tensor_copy(new[0:64, 0:BL:2], ps[:, 0:BL:2])
                    nc.vector.tensor_copy(new[64:128, 1:BL:2], ps[:, 1:BL:2])

                # stream this chunk's states out on the spare gpsimd ring
                nc.gpsimd.dma_start(
                    out=states[:, sc0 * BL : (sc0 + wc) * BL],
                    in_=st[:, sc0 * BL : (sc0 + wc) * BL],
                )

    return nc


_NC_CACHE = None


def _get_nc():
    global _NC_CACHE
    if _NC_CACHE is None:
        _NC_CACHE = _build_nc()
        _NC_CACHE.finalize()
    return _NC_CACHE


def _assign_rows(lengths):
    """Greedy length-balanced row -> core assignment (16 rows per core).

    Returns perm with perm[c*BL + i] = global row index handled by core
    c at local slot i.
    """
    cost = [sum(w for w, s0 in zip(CHUNKS, STARTS) if L > s0) for L in lengths]
    order = sorted(range(len(lengths)), key=lambda g: -cost[g])
    loads = [0.0] * NCORES
    members = [[] for _ in range(NCORES)]
    for g in order:
        c = min(
            (c for c in range(NCORES) if len(members[c]) < BL),
            key=lambda c: loads[c],
        )
        members[c].append(g)
        loads[c] += cost[g]
    # within a core, keep rows sorted by length (desc) so each pair
    # (2j, 2j+1) has similar lengths -> per-pair skip flags lose little
    for c in range(NCORES):
        members[c].sort(key=lambda g: -cost[g])
    return [g for c in range(NCORES) for g in members[c]]


def _make_in_maps(scores, targets, lengths):
    scores = np.ascontiguousarray(np.asarray(scores, dtype=np.float32))
    targets = np.asarray(targets).astype(np.int64)
    lengths = np.asarray(lengths).astype(np.int64)

    perm = _assign_rows([int(x) for x in lengths])

    in_maps = []
    for c in range(NCORES):
        rows = perm[c * BL : (c + 1) * BL]
        sc_shard = np.ascontiguousarray(scores[rows])
        tg = targets[rows]        # [BL, T]
        ln = lengths[rows]        # [BL]

        # element index into flattened local scores [BL*T*K*K]
        b_idx = np.arange(BL)[:, None]
        t_idx = np.arange(T)[None, :]
        flat = (b_idx * T + t_idx) * (K * K) + tg  # [BL, T]
        valid = t_idx < ln[:, None]  # [BL, T]
        flat = np.where(valid, flat, np.int64(0x7FFFFF00))
        gidx = flat.reshape(128, G).astype(np.int32)

        fl = np.zeros((1, NCHUNK * NPAIR), dtype=np.int32)
        for j in range(NPAIR):
            lmax = max(int(ln[2 * j]), int(ln[2 * j + 1]))
            for cc, s0 in enumerate(STARTS):
                fl[0, cc * NPAIR + j] = 1 if lmax > s0 else 0

        in_maps.append(
            {
                "scores": sc_shard,
                "gidx": np.ascontiguousarray(gidx),
                "flags": fl,
            }
        )
    return in_maps, lengths, perm


def _combine(results, lengths, perm):
    all_scores = 0.0
    gold_total = 0.0
    for c in range(NCORES):
        stv = np.asarray(results[c]["states"], dtype=np.float32)  # [128, T*BL]
        gold_total += float(
            np.asarray(results[c]["gold"], dtype=np.float32).sum()
        )
        for bl in range(BL):
            g = perm[c * BL + bl]
            L = int(lengths[g])
            tau = L - 1
            a_end = float(stv[(bl % 2) * 64 + END, tau * BL + bl])
            all_scores += math.log(a_end) + tau * LOG_C
    return np.float32((all_scores - gold_total) / B)


def kernel(scores, targets, lengths, trace=False):
    nc = _get_nc()
    in_maps, ln, perm = _make_in_maps(scores, targets, lengths)
    res = run_bass_kernel_spmd(
        nc, in_maps, core_ids=list(range(NCORES)), trace=trace
    )
    out = _combine(res.results, ln, perm)
    if trace:
        return out, res
    return out
